# revision 9
# baseline (speedup 1.0000x reference)
"""Linear-attention + LePE depthwise-conv kernel for Trainium2 (8 NeuronCores).

Problem (per batch b of 8, one batch per core):
  kv = x_kv @ Wkv + bkv ; k, v = split(kv)
  q = elu(x_q @ Wq + bq) + 1 ; k = elu(k) + 1           (feature maps)
  per head h (8 heads, d = 64):
    k_mean = mean_n k ; z = 1 / (q . k_mean + 1e-6)
    kv_state = (k^T v) / N ; attn = (q @ kv_state) * z
  out = attn + depthwise_conv1d(v, lepe_w, pad=1) + lepe_b

Design notes:
  - Data parallel over batch: core i handles batch i. No collectives.
  - Projections contract over C, so x^T (C on partitions) is required.  x is
    cast f32->bf16 into a DRAM staging buffer (SWDGE cast-DMA), then loaded
    with the DMA xbar transpose (2-byte dtype, DRAM source allows big
    transfers).
  - elu(x)+1 == max(min(e^x, 1), x + 1) exactly.  The "+1" (and the bias) is
    folded into the projection matmul as a rank-1 K=1 term, so the epilogue is
    one ACT exp + one gpsimd min + one DVE max per tile.
  - Token interleave t = 32*p + i (p partition, i tile index): the depthwise
    conv's t+-1 shifts become free-dim shifts of the v tile (plus one tiny
    edge DMA per direction); kv_state is shift-invariant (sum over all t).
  - kv_state and k_sum accumulate in one PSUM tile per head pair, laid out
    block-diagonally so the pass-B matmul computes attn and the z denominator
    for two heads in a single K=128 matmul.
"""

import numpy as np

import concourse.bass as bass
import concourse.mybir as mybir
import concourse.tile as tile
from concourse import bacc
from concourse.bass_utils import run_bass_kernel_spmd

F32 = mybir.dt.float32
BF16 = mybir.dt.bfloat16
AF = mybir.ActivationFunctionType
OP = mybir.AluOpType

B, N, C = 8, 4096, 512
H, D = 8, 64
NTILES = 32          # i in t = 32*p + i
P = 128
CB = C // P          # 4 c-in blocks
NCHUNK = 512         # free-dim chunk for T-layout projections
INV_N = 1.0 / N


def build_program(lepe_b_nonzero: bool, mode: str = "full"):
    nc = bacc.Bacc(None, target_bir_lowering=False, debug=False, num_devices=B)

    x_q = nc.dram_tensor("x_q", [N, C], F32, kind="ExternalInput").ap()
    x_kv = nc.dram_tensor("x_kv", [N, C], F32, kind="ExternalInput").ap()
    Wq = nc.dram_tensor("Wq", [C, C], F32, kind="ExternalInput").ap()
    bq = nc.dram_tensor("bq", [C], F32, kind="ExternalInput").ap()
    Wkv = nc.dram_tensor("Wkv", [C, 2 * C], F32, kind="ExternalInput").ap()
    bkv = nc.dram_tensor("bkv", [2 * C], F32, kind="ExternalInput").ap()
    lepe_w = nc.dram_tensor("lepe_w", [C, 1, 3], F32, kind="ExternalInput").ap()
    lepe_b = nc.dram_tensor("lepe_b", [C], F32, kind="ExternalInput").ap()
    out = nc.dram_tensor("out", [N, C], F32, kind="ExternalOutput").ap()

    with tile.TileContext(nc) as tc:
        with (
            tc.tile_pool(name="per", bufs=1) as per,          # persistent SBUF
            tc.tile_pool(name="inp", bufs=1) as inp,          # x^T tiles
            tc.tile_pool(name="tr", bufs=3) as tr,            # transient SBUF
            tc.tile_pool(name="dram", bufs=1, space="DRAM") as dram,
        ):
            # ---------------- constants / weights prep ----------------
            wq_bf = per.tile([P, CB, C], BF16, tag="wq")
            nc.gpsimd.dma_start(
                out=wq_bf[:], in_=Wq.rearrange("(cb q) co -> q cb co", q=P))
            wkv_bf = per.tile([P, CB, 2 * C], BF16, tag="wkv")
            nc.gpsimd.dma_start(
                out=wkv_bf[:], in_=Wkv.rearrange("(cb q) co -> q cb co", q=P))

            brow_f = per.tile([1, 2 * C + C], F32, tag="brow_f")  # bkv | bq
            nc.sync.dma_start(out=brow_f[:, 0:2 * C], in_=bkv[None, :])
            nc.sync.dma_start(out=brow_f[:, 2 * C:], in_=bq[None, :])
            bk1 = per.tile([1, C], BF16, tag="bk1")
            nc.vector.tensor_scalar_add(bk1[:], brow_f[:, 0:C], 1.0)
            bv = per.tile([1, C], BF16, tag="bv")
            nc.vector.tensor_copy(bv[:], brow_f[:, C:2 * C])
            bq1 = per.tile([1, C], BF16, tag="bq1")
            nc.vector.tensor_scalar_add(bq1[:], brow_f[:, 2 * C:], 1.0)

            ones_k = per.tile([1, P], BF16, tag="ones_k")       # K=1 lhsT
            nc.vector.memset(ones_k[:], 1.0)
            ones_n = per.tile([1, NCHUNK], BF16, tag="ones_n")  # K=1 rhs
            nc.vector.memset(ones_n[:], 1.0)
            ones_col = per.tile([P, 1], BF16, tag="ones_col")   # k_sum rhs
            nc.vector.memset(ones_col[:], 1.0)
            negone = per.tile([P, 1], F32, tag="negone")        # exp bias
            nc.vector.memset(negone[:], -1.0)

            # lepe taps, replicated across partitions
            wtap_f = per.tile([1, 3, C], F32, tag="wtap_f")
            for j in range(3):
                nc.sync.dma_start(out=wtap_f[:, j, :], in_=lepe_w[None, :, 0, j])
            wtap_b = per.tile([1, 3, C], BF16, tag="wtap_b")
            nc.vector.tensor_copy(wtap_b[:], wtap_f[:])
            w_rep = per.tile([P, 3, C], BF16, tag="w_rep")
            for j in range(3):
                nc.gpsimd.partition_broadcast(w_rep[:, j, :], wtap_b[:, j, :])
            if lepe_b_nonzero:
                lb_f = per.tile([1, C], F32, tag="lb_f")
                nc.sync.dma_start(out=lb_f[:], in_=lepe_b[None, :])
                lb_rep = per.tile([P, C], F32, tag="lb_rep")
                nc.gpsimd.partition_broadcast(lb_rep[:], lb_f[:])

            # ---------------- stage 1: cast + transpose loads ----------------
            xkv_bf = dram.tile([N, C], BF16, tag="xkv_bf")
            xq_bf = dram.tile([N, C], BF16, tag="xq_bf")
            for t4 in range(4):
                sl = slice(t4 * (N // 4), (t4 + 1) * (N // 4))
                nc.gpsimd.dma_start(out=xkv_bf[sl, :], in_=x_kv[sl, :])
            for t4 in range(4):
                sl = slice(t4 * (N // 4), (t4 + 1) * (N // 4))
                nc.gpsimd.dma_start(out=xq_bf[sl, :], in_=x_q[sl, :])

            xkvT = inp.tile([P, CB, N], BF16, tag="xkvT")
            xqT = inp.tile([P, CB, N], BF16, tag="xqT")
            for cb in range(CB):
                nc.sync.dma_start(
                    out=xkvT[:, cb, :], in_=xkv_bf[:, cb * P:(cb + 1) * P],
                    transpose=True)
            for cb in range(CB):
                nc.sync.dma_start(
                    out=xqT[:, cb, :], in_=xq_bf[:, cb * P:(cb + 1) * P],
                    transpose=True)

            # token-interleaved views: t = 32*p + i
            xkvT_w = xkvT[:].rearrange("q cb (p i) -> q cb i p", i=NTILES)
            xqT_w = xqT[:].rearrange("q cb (p i) -> q cb i p", i=NTILES)

            v3 = per.tile([P, NTILES, C], BF16, tag="v3")
            qT = per.tile([P, CB, N], BF16, tag="qT")

            with (
                tc.tile_pool(name="psA", bufs=2, space="PSUM") as psA,
                tc.tile_pool(name="pskv", bufs=1, space="PSUM") as pskv,
            ):
                kvp = [pskv.tile([P, 130], F32, tag=f"kv{pp}", name=f"kv{pp}") for pp in range(4)]

                # ------------ stage 2: k/v projection + elu + kv_state ------------
                for i in range(NTILES):
                    pk = psA.tile([P, C], F32, tag="proj")
                    pv = psA.tile([P, C], F32, tag="pv")
                    for cb in range(CB):
                        nc.tensor.matmul(
                            pk[:], xkvT_w[:, cb, i, :], wkv_bf[:, cb, 0:C],
                            start=(cb == 0), stop=False)
                    nc.tensor.matmul(pk[:], ones_k[:], bk1[:],
                                     start=False, stop=True)  # + bias + 1
                    for cb in range(CB):
                        nc.tensor.matmul(
                            pv[:], xkvT_w[:, cb, i, :], wkv_bf[:, cb, C:2 * C],
                            start=(cb == 0), stop=False)
                    nc.tensor.matmul(pv[:], ones_k[:], bv[:],
                                     start=False, stop=True)  # + bias

                    # elu(y)+1 = max(min(e^y, 1), y+1); pk holds y+1
                    ek = tr.tile([P, C], BF16, tag="ek")
                    nc.scalar.activation(ek[:], pk[:], AF.Exp, bias=negone[:])
                    mk = tr.tile([P, C], BF16, tag="mk")
                    nc.gpsimd.tensor_scalar_min(mk[:], ek[:], 1.0)
                    kb = tr.tile([P, C], BF16, tag="kb")
                    nc.vector.tensor_tensor(kb[:], mk[:], pk[:], op=OP.max)

                    nc.scalar.copy(v3[:, i, :], pv[:])
                    if mode == "k":
                        kf = tr.tile([P, C], F32, tag="kf")
                        nc.vector.tensor_copy(kf[:], kb[:])
                        nc.sync.dma_start(
                            out=out.rearrange("(i2 p) c -> i2 p c", p=P)[i],
                            in_=kf[:])

                    last = (i == NTILES - 1)
                    # One start=True per PSUM bank: start clears the whole
                    # bank's has_written bits, so the other three region
                    # groups must never set it (their first write lands on
                    # has_written=0 and overwrites, which is the correct
                    # init).
                    for pp in range(4):
                        c0 = pp * P
                        nc.tensor.matmul(
                            kvp[pp][0:64, 0:64], kb[:, c0:c0 + 64],
                            v3[:, i, c0:c0 + 64], start=(i == 0), stop=last,
                            skip_group_check=True)
                        nc.tensor.matmul(
                            kvp[pp][0:64, 64:65], kb[:, c0:c0 + 64],
                            ones_col[:], start=False, stop=last,
                            skip_group_check=True)
                        nc.tensor.matmul(
                            kvp[pp][64:128, 65:129], kb[:, c0 + 64:c0 + 128],
                            v3[:, i, c0 + 64:c0 + 128], start=False, stop=last,
                            skip_group_check=True)
                        nc.tensor.matmul(
                            kvp[pp][64:128, 129:130], kb[:, c0 + 64:c0 + 128],
                            ones_col[:], start=False, stop=last,
                            skip_group_check=True)

                # ------------ stage 3: q projection (T layout) + elu ------------
                for a in range(CB):
                    for tch in range(N // NCHUNK):
                        ts = slice(tch * NCHUNK, (tch + 1) * NCHUNK)
                        pq = psA.tile([P, NCHUNK], F32, tag="proj")
                        for cb in range(CB):
                            nc.tensor.matmul(
                                pq[:], wq_bf[:, cb, a * P:(a + 1) * P],
                                xqT[:, cb, ts], start=(cb == 0), stop=False)
                        nc.tensor.matmul(
                            pq[:], bq1[:, a * P:(a + 1) * P], ones_n[:],
                            start=False, stop=True)  # + bias + 1
                        eq = tr.tile([P, NCHUNK], BF16, tag="ek")
                        nc.scalar.activation(eq[:], pq[:], AF.Exp, bias=negone[:])
                        mq = tr.tile([P, NCHUNK], BF16, tag="mk")
                        nc.gpsimd.tensor_scalar_min(mq[:], eq[:], 1.0)
                        nc.vector.tensor_tensor(qT[:, a, ts], mq[:], pq[:],
                                                op=OP.max)
                        if mode == "q":
                            qf = tr.tile([P, NCHUNK], F32, tag="kf")
                            nc.vector.tensor_copy(qf[:], qT[:, a, ts])
                            nc.sync.dma_start(
                                out=out.rearrange("(a2 t2 p) c -> a2 t2 p c", a2=CB, p=P)[a, tch],
                                in_=qf[:])

                # ------------ stage 4: kv_aug assembly + conv edge tiles --------
                kv_aug = [per.tile([P, 130], BF16, tag=f"kva{pp}", name=f"kva{pp}")
                          for pp in range(4)]
                for pp in range(4):
                    nc.vector.memset(kv_aug[pp][:], 0.0)
                    nc.scalar.mul(kv_aug[pp][0:64, 0:65],
                                  kvp[pp][0:64, 0:65], INV_N)
                    nc.scalar.mul(kv_aug[pp][64:128, 65:130],
                                  kvp[pp][64:128, 65:130], INV_N)

            if mode == "kv":
                for pp in range(4):
                    kvf = per.tile([P, 130], F32, tag=f"kvf{pp}", name=f"kvf{pp}")
                    nc.vector.tensor_copy(kvf[:], kv_aug[pp][:])
                    nc.sync.dma_start(out=out[128 * pp:128 * (pp + 1), 0:130],
                                      in_=kvf[:])
            em1 = per.tile([P, C], BF16, tag="em1")  # v[32p-1]
            nc.vector.memset(em1[0:1, :], 0.0)
            nc.sync.dma_start(out=em1[1:128, :], in_=v3[0:127, NTILES - 1, :])
            ep1 = per.tile([P, C], BF16, tag="ep1")  # v[32p+32]
            nc.vector.memset(ep1[:], 0.0)
            nc.sync.dma_start(out=ep1[0:127, :], in_=v3[1:128, 0, :])

            qT_w = qT[:].rearrange("q a (p i) -> q a i p", i=NTILES)
            out_w = out.rearrange("(p i) c -> i p c", i=NTILES)

            # ------------ stage 5: attention out + z + lepe ------------
            with tc.tile_pool(name="psB", bufs=2, space="PSUM") as psB:
                for i in range(NTILES):
                    po = [psB.tile([P, 130], F32, tag=f"o{pp}", name=f"o{pp}")
                          for pp in range(4)]
                    for pp in range(4):
                        nc.tensor.matmul(po[pp][:], qT_w[:, pp, i, :],
                                         kv_aug[pp][:], start=True, stop=True)
                    den = tr.tile([P, 8], F32, tag="den")
                    for pp in range(4):
                        nc.vector.tensor_copy(
                            den[:, 2 * pp:2 * pp + 2],
                            po[pp][:].rearrange("p (g c) -> p g c", g=2)[:, :, 64])
                    if mode == "den":
                        nc.sync.dma_start(out=out_w[i][:, 0:8], in_=den[:])
                    rec = tr.tile([P, 8], F32, tag="rec")
                    nc.vector.tensor_scalar_add(den[:], den[:], 1e-6)
                    nc.vector.reciprocal(rec[:], den[:])

                    osb = tr.tile([P, C], F32, tag="osb")
                    for h in range(H):
                        pp, s = h // 2, h % 2
                        nc.vector.tensor_scalar(
                            osb[:, 64 * h:64 * h + 64],
                            po[pp][:, 65 * s:65 * s + 64],
                            rec[:, h:h + 1], None, op0=OP.mult)

                    # lepe: w0*v[t-1] + w1*v[t] + w2*v[t+1]
                    vm = em1[:] if i == 0 else v3[:, i - 1, :]
                    vc = v3[:, i, :]
                    vp = ep1[:] if i == NTILES - 1 else v3[:, i + 1, :]
                    t0 = tr.tile([P, C], BF16, tag="t0")
                    nc.gpsimd.tensor_tensor(t0[:], w_rep[:, 0, :], vm, op=OP.mult)
                    t1 = tr.tile([P, C], BF16, tag="t1")
                    nc.vector.tensor_tensor(t1[:], w_rep[:, 1, :], vc, op=OP.mult)
                    t2 = tr.tile([P, C], BF16, tag="t2")
                    nc.gpsimd.tensor_tensor(t2[:], w_rep[:, 2, :], vp, op=OP.mult)
                    s01 = tr.tile([P, C], BF16, tag="s01")
                    nc.vector.tensor_tensor(s01[:], t0[:], t1[:], op=OP.add)
                    s012 = tr.tile([P, C], BF16, tag="s012")
                    nc.gpsimd.tensor_tensor(s012[:], s01[:], t2[:], op=OP.add)
                    if mode == "lepe":
                        nc.vector.tensor_copy(osb[:], s012[:])
                    elif mode != "attn":
                        nc.vector.tensor_tensor(osb[:], osb[:], s012[:], op=OP.add)
                    if lepe_b_nonzero:
                        nc.vector.tensor_tensor(osb[:], osb[:], lb_rep[:],
                                                op=OP.add)

                    if mode in ("full", "attn", "lepe"):
                        nc.sync.dma_start(out=out_w[i], in_=osb[:])

    nc.finalize()
    return nc


_CACHE = {}


def kernel(x_q, x_kv, Wq, bq, Wkv, bkv, lepe_w, lepe_b):
    x_q = np.asarray(x_q, dtype=np.float32)
    x_kv = np.asarray(x_kv, dtype=np.float32)
    Wq = np.asarray(Wq, dtype=np.float32)
    bq = np.asarray(bq, dtype=np.float32)
    Wkv = np.asarray(Wkv, dtype=np.float32)
    bkv = np.asarray(bkv, dtype=np.float32)
    lepe_w = np.asarray(lepe_w, dtype=np.float32)
    lepe_b = np.asarray(lepe_b, dtype=np.float32)

    lb_nz = bool(np.any(lepe_b))
    if lb_nz not in _CACHE:
        _CACHE[lb_nz] = build_program(lb_nz)
    nc = _CACHE[lb_nz]

    in_maps = [
        dict(x_q=np.ascontiguousarray(x_q[b]), x_kv=np.ascontiguousarray(x_kv[b]),
             Wq=Wq, bq=bq, Wkv=Wkv, bkv=bkv, lepe_w=lepe_w, lepe_b=lepe_b)
        for b in range(B)
    ]
    res = run_bass_kernel_spmd(nc, in_maps, core_ids=list(range(B)))
    return np.stack([res.results[b]["out"] for b in range(B)], axis=0)
